# revision 30
# baseline (speedup 1.0000x reference)
"""2-layer GCN encoder (PyG GCNConv semantics) on 8 Trainium2 NeuronCores.

Strategy (dst-sharded graph parallel, v2):
- Nodes are dst-sharded: a degree-sorted deal across the 8 cores balances
  per-core edge counts; within each core half, a (d0,d1)-lex sort packs
  similar in-degrees into the same 128-dst tile (tight gather rectangles).
  The gathered g-table is TILE-MAJOR (row = chunk*7168 + core*896 + j%896) so
  each 7-tile AllGather chunk is a contiguous table slice, letting the
  collectives overlap compute (AG1 chunks fire during the layer-1 matmuls,
  AG2 chunks one group behind the layer-2 matmuls inside aggregation).
- GCN norm is separable: norm(e) = dinv[src]*dinv[dst].  dinv[src] is folded
  into featT on the host, so layer-1 matmuls produce g = (x*dinv) @ W1
  directly.  Each layer's aggregation dma_gathers source rows per 4-tile
  group (slots padded to per-tile max in-degree; low/high table halves keep
  indices in int16; pads spread over ~176 spare zero rows so they never
  serialize one HBM address; calls spread over 4 SWDGE queues), tree-reduces
  on the Vector engine, adds the locally-computed self-loop term, and applies
  relu((acc+own)*dinv^2) on the Scalar engine (layer 2: *dinv, DMA out).
  Self-loop edges never hit the gather.  Tables are Internal+Shared DRAM so
  the AllGather takes the shared-output fast path (each core writes only its
  1/8 shard).
- Host does index/layout preprocessing only (sorting, slot assignment, feat
  transpose+scale); all FLOPs and feature movement run on device.
"""
import sys
import os

for _p in ("/opt/trn_rl_repo", "/root/.axon_site/_ro/trn_rl_repo"):
    if os.path.isdir(_p) and _p not in sys.path:
        sys.path.insert(0, _p)

import numpy as np
import concourse.bass as bass
import concourse.bacc as bacc
import concourse.tile as tile
import concourse.mybir as mybir
from concourse.masks import make_identity
from concourse.bass_utils import run_bass_kernel_spmd

F32 = mybir.dt.float32
I16 = mybir.dt.int16
RELU = mybir.ActivationFunctionType.Relu

N_NODES = 50000
IN_DIM = 256
OUT_DIM = 64
N_CORES = 8
TILES = 49                  # ceil(6250/128)
SH = TILES * 128            # 6272 rows per core shard (incl. 22 zero spares)
CHUNK = 7                   # tiles per AllGather chunk (49 = 7x7)
CHROWS = CHUNK * 128 * N_CORES   # 7168 table rows per chunk (tile-major)
# Tile-major table: row(c, j) = (j//896)*7168 + c*896 + j%896.  The low half
# is chunks 0..3 (j < 3584, rows < 28672 <= int16 range); spare zero rows sit
# at the end of each half's last tile so both halves have pad targets.
LOWJ = CHUNK * 128 * 4      # 3584: per-core j-extent of the low half
SPLIT = 4 * CHROWS          # 28672 table rows in the low half
N_SP_LO = 12                # spare (zero) j-slots at the end of the low half
N_SP_HI = 10                # spare j-slots at the end of the high half
SMAX = 8                    # max slots per dma_gather call (HW ring limit)
MSG_BUFS = 3                # message buffers (one per GROUP-tile group)
GROUP = 4                   # dst tiles per msg buffer / gather-call stream
N_QUEUES = 4


def _host_prep(feat, W1, b1, W2, b2, edge_index):
    N, C, T = N_NODES, N_CORES, TILES
    src0 = np.asarray(edge_index[0], dtype=np.int64)
    dst0 = np.asarray(edge_index[1], dtype=np.int64)
    loops = np.arange(N, dtype=np.int64)
    # deg matches the reference (self-loops included in the normalization)
    deg = (np.bincount(np.concatenate([dst0, loops]), minlength=N)
           .astype(np.int64))

    # deal degree-sorted nodes across cores (balances per-core edge counts
    # and aligns tile degree profiles across cores)
    order0 = np.argsort(deg, kind="stable")
    core = np.empty(N, np.int64)
    core[order0] = np.arange(N) % C

    # half membership: within each core, the lowest-degree LOWJ-N_SP_LO nodes
    # form the "low" half of the table (chunks 0..3), the rest the high half
    n_low_real = LOWJ - N_SP_LO
    is_low_node = np.zeros(N, bool)
    for c in range(C):
        nodes_c = np.where(core == c)[0]
        o = nodes_c[np.argsort(deg[nodes_c], kind="stable")]
        is_low_node[o[:n_low_real]] = True

    # self-loops are computed locally, so only real edges hit the gather
    src, dst = src0, dst0
    is_low = is_low_node[src]
    d0 = np.bincount(dst[is_low], minlength=N)
    d1 = np.bincount(dst[~is_low], minlength=N)

    # (d0, d1)-lex order within each core half -> local slot j
    j = np.empty(N, np.int64)
    for c in range(C):
        nodes_c = np.where(core == c)[0]
        lo = nodes_c[is_low_node[nodes_c]]
        hi = nodes_c[~is_low_node[nodes_c]]
        olo = lo[np.lexsort((d1[lo], d0[lo]))]
        ohi = hi[np.lexsort((d1[hi], d0[hi]))]
        j[olo] = np.arange(len(olo))
        j[ohi] = LOWJ + np.arange(len(ohi))

    def rowmap(c, jj):
        # tile-major global table row
        return (jj // 896) * CHROWS + c * 896 + jj % 896

    row = rowmap(core, j)

    # per-tile slot maxes (shared across cores so the program is SPMD)
    S0 = np.zeros(T, np.int64)
    S1 = np.zeros(T, np.int64)
    tl = j // 128
    for t in range(T):
        m = tl == t
        if m.any():
            S0[t] = d0[m].max()
            S1[t] = d1[m].max()

    # edge -> slot assignment (per dst, low edges then high; sources sorted
    # ascending within each dst so a (tile, slot) gather call touches a
    # narrow quantile band of the table -> DRAM row locality)
    dkey = core[dst] * SH + j[dst]
    e_order = np.lexsort((row[src], (~is_low).astype(np.int64), dkey))
    es, ed, el = src[e_order], dst[e_order], is_low[e_order]
    key = dkey[e_order] * 2 + (~el).astype(np.int64)
    occ = np.zeros(len(es), np.int64)
    _, first_idx, counts = np.unique(key, return_index=True, return_counts=True)
    for fi, cnt in zip(first_idx, counts):
        occ[fi:fi + cnt] = np.arange(cnt)

    iA = np.full((C, T, 128, max(1, int(S0.max()))), -1, np.int64)
    iB = np.full((C, T, 128, max(1, int(S1.max()))), -1, np.int64)
    ec = core[ed]
    ep = j[ed] % 128
    et = j[ed] // 128
    lm = el
    iA[ec[lm], et[lm], ep[lm], occ[lm]] = row[es[lm]]
    hm = ~el
    iB[ec[hm], et[hm], ep[hm], occ[hm]] = row[es[hm]] - SPLIT

    # Spread pad targets over every spare zero row of the half (~37k pad
    # descs/layer on a single 256B address would serialize one HBM channel).
    spareA = np.concatenate([rowmap(c, np.arange(LOWJ - N_SP_LO, LOWJ))
                             for c in range(C)])
    spareB = np.concatenate([rowmap(c, np.arange(SH - N_SP_HI, SH))
                             for c in range(C)]) - SPLIT
    padA = iA < 0
    iA[padA] = spareA[np.arange(int(padA.sum())) % len(spareA)]
    padB = iB < 0
    iB[padB] = spareB[np.arange(int(padB.sum())) % len(spareB)]

    def wrap16(v):
        # idx position j -> [j%16, j//16], replicated across the 8 Q7 cores
        w = v.reshape(-1, 16).T.astype(np.int16)
        return np.tile(w, (8, 1))

    percore_idx = []
    for c in range(C):
        colsA, colsB = [], []
        for t in range(T):
            if S0[t] > 0:
                colsA.append(iA[c, t, :, :S0[t]].T.reshape(-1))
            if S1[t] > 0:
                colsB.append(iB[c, t, :, :S1[t]].T.reshape(-1))
        vA = np.concatenate(colsA) if colsA else np.zeros(16, np.int64)
        vB = np.concatenate(colsB) if colsB else np.zeros(16, np.int64)
        percore_idx.append((wrap16(vA), wrap16(vB)))

    dinv_full = 1.0 / np.sqrt(deg.astype(np.float64))

    featT = np.zeros((C, IN_DIM, SH), np.float32)
    dinvt = np.zeros((C, 128, T), np.float32)   # spares stay 0 -> rows stay 0
    feat = np.asarray(feat, np.float32)
    for c in range(C):
        nodes_c = np.where(core == c)[0]
        featT[c][:, j[nodes_c]] = (feat[nodes_c] * dinv_full[nodes_c, None]).T
        dinvt[c, j[nodes_c] % 128, j[nodes_c] // 128] = dinv_full[nodes_c]

    W1 = np.asarray(W1, np.float32)
    W2 = np.asarray(W2, np.float32)
    b1 = np.asarray(b1, np.float32)
    b2 = np.asarray(b2, np.float32)
    has_bias = bool(np.any(b1 != 0) or np.any(b2 != 0))
    in_maps = []
    for c in range(C):
        in_maps.append({
            "featT": featT[c],
            "idxA": np.ascontiguousarray(percore_idx[c][0]),
            "idxB": np.ascontiguousarray(percore_idx[c][1]),
            "dinvt": dinvt[c],
            "dinv2t": (dinvt[c] * dinvt[c]).astype(np.float32),
            "W1": W1.reshape(2, 128, OUT_DIM),
            "W2": W2,
            "b1": np.broadcast_to(b1, (128, OUT_DIM)).copy(),
            "b2": np.broadcast_to(b2, (128, OUT_DIM)).copy(),
        })
    post = {"core": core, "j": j}
    return in_maps, S0.astype(int), S1.astype(int), has_bias, post


def _build_nc(S0, S1, has_bias=False, reps=1):
    C, T, D = N_CORES, TILES, OUT_DIM
    KIN = IN_DIM // 128
    CA = int(sum(S0)) * 8
    CB = int(sum(S1)) * 8
    nc = bacc.Bacc(None, target_bir_lowering=False, num_swdge_queues=N_QUEUES)
    featT = nc.dram_tensor("featT", [IN_DIM, SH], F32, kind="ExternalInput")
    idxA = nc.dram_tensor("idxA", [128, max(CA, 16)], I16, kind="ExternalInput")
    idxB = nc.dram_tensor("idxB", [128, max(CB, 16)], I16, kind="ExternalInput")
    dinvt_d = nc.dram_tensor("dinvt", [128, T], F32, kind="ExternalInput")
    dinv2t_d = nc.dram_tensor("dinv2t", [128, T], F32, kind="ExternalInput")
    W1 = nc.dram_tensor("W1", [KIN, 128, D], F32, kind="ExternalInput")
    W2 = nc.dram_tensor("W2", [D, D], F32, kind="ExternalInput")
    b1 = nc.dram_tensor("b1", [128, D], F32, kind="ExternalInput")
    b2 = nc.dram_tensor("b2", [128, D], F32, kind="ExternalInput")
    out = nc.dram_tensor("out", [SH, D], F32, kind="ExternalOutput")

    ag_in = [nc.dram_tensor(f"agin{l}", [SH, D], F32, kind="Internal")
             for l in range(2)]
    table = [nc.dram_tensor(f"table{l}", [C * SH, D], F32, kind="Internal",
                            addr_space="Shared") for l in range(2)]

    with tile.TileContext(nc) as tc:
        with (
            tc.tile_pool(name="const", bufs=1) as constp,
            tc.tile_pool(name="ft", bufs=4) as ftp,
            tc.tile_pool(name="gz", bufs=4) as gzp,
            tc.tile_pool(name="msga", bufs=MSG_BUFS) as msgap,
            tc.tile_pool(name="ps", bufs=4, space="PSUM") as psp,
            tc.tile_pool(name="pst", bufs=2, space="PSUM") as pstp,
        ):
            w1s = []
            for k in range(KIN):
                w1k = constp.tile([128, D], F32, name=f"w1{k}")
                nc.sync.dma_start(out=w1k[:], in_=W1[k, :, :])
                w1s.append(w1k)
            w2 = constp.tile([D, D], F32)
            nc.sync.dma_start(out=w2[:], in_=W2[:, :])
            b1t = constp.tile([128, D], F32)
            nc.sync.dma_start(out=b1t[:], in_=b1[:, :])
            b2t = constp.tile([128, D], F32)
            nc.sync.dma_start(out=b2t[:], in_=b2[:, :])
            ia = constp.tile([128, max(CA, 16)], I16)
            nc.sync.dma_start(out=ia[:], in_=idxA[:, :])
            ib = constp.tile([128, max(CB, 16)], I16)
            nc.sync.dma_start(out=ib[:], in_=idxB[:, :])
            dinv = constp.tile([128, T], F32)
            nc.sync.dma_start(out=dinv[:], in_=dinvt_d[:, :])
            dinv2 = constp.tile([128, T], F32)
            nc.sync.dma_start(out=dinv2[:], in_=dinv2t_d[:, :])
            ident = constp.tile([128, 128], F32)
            make_identity(nc, ident[:])
            z1T = constp.tile([D, SH], F32)
            g_own = [constp.tile([128, T * D], F32, name=f"gown{l}")
                     for l in range(2)]

            qn = [0]
            regs = {}

            def nreg(n):
                if n not in regs:
                    regs[n] = nc.gpsimd.to_reg(n)
                return regs[n]

            def gather_half(msgt, idxt, col0, S_t, base_view, buf_off):
                s = 0
                while s < S_t:
                    cnt = min(SMAX, S_t - s)
                    n = 128 * cnt
                    dst = msgt[:, (buf_off + s) * D:(buf_off + s + cnt) * D]
                    nc.gpsimd.dma_gather(
                        dst.rearrange("p (s d) -> p s d", d=D),
                        base_view,
                        idxt[:, (col0 + s * 8):(col0 + (s + cnt) * 8)],
                        n, nreg(n), D, elem_step=D,
                        queue_num=qn[0] % N_QUEUES)
                    qn[0] += 1
                    s += cnt

            def mm2_tile(t):
                # layer-2 g: z1' @ W2  (dinv for layer 2 applied at z2 time)
                ps2 = psp.tile([128, D], F32, tag="mm")
                nc.tensor.matmul(ps2[:], lhsT=z1T[:, t * 128:(t + 1) * 128],
                                 rhs=w2[:, :], start=True, stop=True)
                nc.scalar.copy(g_own[1][:, t * D:(t + 1) * D], ps2[:])
                nc.sync.dma_start(out=ag_in[1][t * 128:(t + 1) * 128, :],
                                  in_=g_own[1][:, t * D:(t + 1) * D])

            def tree(msg, off, W):
                # in-place tree reduce of msg slots [off, off+W) -> slot off
                Wc = W
                while Wc > 1:
                    h = Wc // 2
                    nc.vector.tensor_add(
                        msg[:, off * D:(off + h) * D],
                        msg[:, off * D:(off + h) * D],
                        msg[:, (off + Wc - h) * D:(off + Wc) * D])
                    Wc = Wc - h

            def aggregate(tbl, layer, on_group_end=None):
                colA = 0
                colB = 0
                is_last = layer == 1
                dscale = dinv if is_last else dinv2
                bias = b2t if is_last else b1t
                for g0 in range(0, T, GROUP):
                    gts = range(g0, min(g0 + GROUP, T))
                    W0g = sum(int(S0[t]) for t in gts)
                    W1g = sum(int(S1[t]) for t in gts)
                    msg = msgap.tile([128, (W0g + W1g) * D], F32, tag="msga")
                    if W0g > 0:
                        gather_half(msg, ia, colA, W0g, tbl[:, :], 0)
                        colA += W0g * 8
                    if W1g > 0:
                        gather_half(msg, ib, colB, W1g, tbl[SPLIT:, :], W0g)
                        colB += W1g * 8
                    offA = 0
                    offB = W0g
                    for t in gts:
                        s0, s1 = int(S0[t]), int(S1[t])
                        gslice = g_own[layer][:, t * D:(t + 1) * D]
                        zt = gzp.tile([128, D], F32, tag="z")
                        if s0 > 0:
                            tree(msg, offA, s0)
                        if s1 > 0:
                            tree(msg, offB, s1)
                        if s0 > 0 and s1 > 0:
                            nc.vector.tensor_add(msg[:, offA * D:(offA + 1) * D],
                                                 msg[:, offA * D:(offA + 1) * D],
                                                 msg[:, offB * D:(offB + 1) * D])
                        acc_off = offA if s0 > 0 else offB
                        if s0 + s1 == 0:
                            acc_ap = gslice
                        else:
                            # fold in the locally-computed self-loop term
                            nc.vector.tensor_add(
                                msg[:, acc_off * D:(acc_off + 1) * D],
                                msg[:, acc_off * D:(acc_off + 1) * D], gslice)
                            acc_ap = msg[:, acc_off * D:(acc_off + 1) * D]
                        offA += s0
                        offB += s1
                        if has_bias:
                            # layer 1 computes z1' = z1*dinv = relu(dinv2*acc
                            # + b1*dinv); layer 2: z2 = relu(dinv*acc + b2)
                            if is_last:
                                bias_ap = bias[:]
                            else:
                                bb = gzp.tile([128, D], F32, tag="zbb")
                                nc.vector.tensor_scalar_mul(bb[:], bias[:],
                                                            dinv[:, t:t + 1])
                                bias_ap = bb[:]
                            tmp = gzp.tile([128, D], F32, tag="zb")
                            nc.vector.scalar_tensor_tensor(
                                tmp[:], acc_ap, dscale[:, t:t + 1], bias_ap,
                                op0=mybir.AluOpType.mult,
                                op1=mybir.AluOpType.add)
                            nc.scalar.activation(zt[:], tmp[:], RELU)
                        else:
                            nc.scalar.activation(zt[:], acc_ap, RELU,
                                                 scale=dscale[:, t:t + 1])
                        if is_last:
                            nc.sync.dma_start(
                                out=out[t * 128:(t + 1) * 128, :], in_=zt[:])
                        else:
                            pst = pstp.tile([D, 128], F32, tag="tr")
                            nc.tensor.transpose(out=pst[:], in_=zt[:],
                                                identity=ident[:])
                            nc.scalar.copy(z1T[:, t * 128:(t + 1) * 128],
                                           pst[:])
                            mm2_tile(t)
                    if on_group_end is not None:
                        on_group_end(gts[-1])

            def emit_ag(layer, k):
                # AllGather chunk k (tiles 7k..7k+6): each core contributes
                # its [896, 64] slice; tile-major table rows are contiguous
                lo, hi = k * CHUNK * 128, (k + 1) * CHUNK * 128
                nc.gpsimd.collective_compute(
                    "AllGather", mybir.AluOpType.bypass,
                    replica_groups=[list(range(C))],
                    ins=[ag_in[layer][lo:hi, :]],
                    outs=[table[layer][k * CHROWS:(k + 1) * CHROWS, :]],
                )

            def mm1_tile(t):
                # layer-1 g: (feat*dinv) @ W1  (dinv folded into featT on host)
                ps = psp.tile([128, D], F32, tag="mm")
                for k in range(KIN):
                    ftk = ftp.tile([128, 128], F32, tag="ft")
                    nc.sync.dma_start(
                        out=ftk[:],
                        in_=featT[k * 128:(k + 1) * 128,
                                  t * 128:(t + 1) * 128])
                    nc.tensor.matmul(ps[:], lhsT=ftk[:], rhs=w1s[k][:, :],
                                     start=(k == 0), stop=(k == KIN - 1))
                nc.scalar.copy(g_own[0][:, t * D:(t + 1) * D], ps[:])
                nc.sync.dma_start(out=ag_in[0][t * 128:(t + 1) * 128, :],
                                  in_=g_own[0][:, t * D:(t + 1) * D])

            mm1_next = [0]
            ag1_next = [0]

            def mm1_advance(upto, ag_lag):
                # emit mm1 tiles up to `upto`, firing AG1 chunks `ag_lag`
                # tiles behind the emission frontier
                while mm1_next[0] < upto:
                    mm1_tile(mm1_next[0])
                    mm1_next[0] += 1
                while (ag1_next[0] < 7
                       and ag1_next[0] * 7 + 7 + ag_lag <= mm1_next[0]):
                    emit_ag(0, ag1_next[0])
                    ag1_next[0] += 1

            for rep in range(reps):
                mm1_advance(T, 0)
                while ag1_next[0] < 7:
                    emit_ag(0, ag1_next[0])
                    ag1_next[0] += 1
                ag2_next = [0]

                def on_group(done):
                    # fire layer-2 AllGather chunks as the mm2s complete
                    while (ag2_next[0] < 7
                           and ag2_next[0] * 7 + 6 <= done):
                        emit_ag(1, ag2_next[0])
                        ag2_next[0] += 1

                aggregate(table[0], 0, on_group_end=on_group)
                while ag2_next[0] < 7:
                    emit_ag(1, ag2_next[0])
                    ag2_next[0] += 1
                mm1_next[0] = 0
                ag1_next[0] = 0
                if rep + 1 < reps:
                    # overlap the next rep's layer-1 matmuls (and AG1 chunks,
                    # one chunk behind) with this rep's layer-2 aggregation
                    pace = [0]

                    def on_group2(done):
                        pace[0] += GROUP
                        mm1_advance(min(T, pace[0]), CHUNK)

                    aggregate(table[1], 1, on_group_end=on_group2)
                else:
                    aggregate(table[1], 1)

    nc.finalize()
    return nc


def kernel(feat, W1, b1, W2, b2, edge_index, _reps=1, _return_nc=False):
    in_maps, S0, S1, has_bias, post = _host_prep(feat, W1, b1, W2, b2,
                                                 edge_index)
    nc = _build_nc(S0, S1, has_bias=has_bias, reps=_reps)
    if _return_nc:
        return nc, in_maps, post
    res = run_bass_kernel_spmd(nc, in_maps, core_ids=list(range(N_CORES)))
    full = np.empty((N_NODES, OUT_DIM), np.float32)
    core, j = post["core"], post["j"]
    for c in range(N_CORES):
        oc = res.results[c]["out"]
        nodes_c = np.where(core == c)[0]
        full[nodes_c] = oc[j[nodes_c]]
    return full
